# revision 10
# baseline (speedup 1.0000x reference)
"""Trainium2 Bass kernel for nn_AttentionLayer_77558519431766.

Math: the reference computes softmax over a size-1 axis, which is
identically 1.0, so the attention MLP is dead code and

    out[b, e] = sum_{i<j} x[b,i,e] * x[b,j,e]
              = 0.5 * ((sum_f x[b,f,e])^2 - sum_f x[b,f,e]^2)

Implementation v4 (PE reduction pipeline, column-sliced DMA):
  Per 128-row chunk the 50 f-planes stream in as column slices so compute
  starts ~4us in instead of waiting for the whole chunk. Per slice:
    - DVE casts f32 -> bf16 (2x_2p)
    - PE transposes each [128b, 128(f2,e)] block into PSUM
    - DVE + ACT split the PSUM->SBUF copyback (xT for the s-chain);
      ACT squares the transposed blocks into SBUF (sq, scaled by 0.5)
    - PE accumulates s = sum_f x and q = 0.5*sum_f x^2 via one-hot
      stacked masks [128,64], grouped per slice to limit LDWEIGHTS churn
  Chunk combine: res = Square(s*sqrt(.5)) - q, one small DMA out.
  ~30 dummy matmuls at t=0 warm the PE HAM clock gate (1.2 -> 2.4 GHz)
  under the first DMA's shadow.

Sharding: pure data parallelism, batch 2048 -> 8 shards of 256.
"""

import numpy as np

try:
    import concourse.bass as bass  # noqa: F401
except ImportError:  # pragma: no cover
    import sys

    sys.path.insert(0, "/opt/trn_rl_repo")

_B, _F, _E = 2048, 50, 64
_NCORES = 8
_BS = _B // _NCORES  # 256 rows per core
_ROW = _F * _E  # 3200 floats per row
_P = 128  # SBUF partitions
_NCHUNK = _BS // _P  # 2
# f-planes per DMA slice, per chunk (must be even: 1 block = 2 planes).
# Last slice of the last chunk is small to shorten the critical tail.
_SLICES = [
    [6, 14, 14, 16],
    [16, 16, 16, 2],
]
_WARM_MMS = 30  # dummy matmuls to release the PE HAM clock gate


def _make_tc_class():
    """TileContext with a slim kernel tail (drops the redundant tail
    sem-clear + second barrier; the Bass preamble re-clears at start)."""
    from concourse.tile import TileContext
    from concourse.vector_clock import ScopedClock

    class SlimTailTileContext(TileContext):
        def _drain_and_barrier(self, tick_clock, wait_clock):
            drain_inst = self.nc.sync.drain()
            wait_clock.add_sem_waits(
                drain_inst.ins, ScopedClock({None: tick_clock.global_clock})
            )
            self.nc.all_engine_barrier(sem_only=True)
            popped = self.nc._tile_sem_poison_stack.pop()
            assert popped is self._sem_poison

    return SlimTailTileContext


def _build():
    import concourse.bacc as bacc
    import concourse.mybir as mybir

    TileContext = _make_tc_class()

    f32 = mybir.dt.float32
    bf16 = mybir.dt.bfloat16
    i32 = mybir.dt.int32
    SQ = mybir.ActivationFunctionType.Square
    ALU = mybir.AluOpType
    HALF_SQRT = float(np.float32(np.sqrt(0.5)))

    nc = bacc.Bacc()
    x = nc.declare_dram_parameter("inputs", [_BS, _ROW], f32, isOutput=False)
    out = nc.declare_dram_parameter("out", [_NCHUNK, _P, _E], f32, isOutput=True)

    with TileContext(nc) as tc:
        with (
            tc.tile_pool(name="consts", bufs=1) as cpool,
            tc.tile_pool(name="x", bufs=6) as xpool,
            tc.tile_pool(name="xb", bufs=3) as xbpool,
            tc.tile_pool(name="xT", bufs=3) as xtpool,
            tc.tile_pool(name="xsq", bufs=3) as sqpool,
            tc.tile_pool(name="pt", bufs=3, space="PSUM") as ptpool,
            tc.tile_pool(name="acc", bufs=2, space="PSUM") as accpool,
            tc.tile_pool(name="warmps", bufs=1, space="PSUM") as wpool,
            tc.tile_pool(name="small", bufs=1) as spool,
        ):
            # ACT warm op: hoists the Square function-table load.
            warm = spool.tile([_P, 1], f32, tag="warm")
            nc.gpsimd.memset(warm[:], 0.0)
            nc.scalar.activation(warm[:], warm[:], SQ)

            # Constants built on-chip. iota with channel_multiplier=-1
            # gives v[p,j] = j - p; the stacked one-hot mask [128,64] has
            # ones where j - p is 0 or -64. Masks are pre-scaled so the
            # final combine needs no extra scaling: s-chain mask
            # sqrt(0.5)-one-hot -> s'^2 = 0.5 s^2; q-chain 0.5-one-hot.
            iot_i = cpool.tile([_P, _P], i32, tag="iot_i")
            iot_m = cpool.tile([_P, _E], i32, tag="iot_m")
            ident = cpool.tile([_P, _P], bf16, tag="ident")
            mask = cpool.tile([_P, _E], bf16, tag="mask")
            mask_b = cpool.tile([_P, _E], bf16, tag="mask_b")
            nc.gpsimd.iota(iot_i[:], pattern=[[1, _P]], base=0, channel_multiplier=-1)
            nc.gpsimd.iota(iot_m[:], pattern=[[1, _E]], base=0, channel_multiplier=-1)
            nc.vector.tensor_scalar(ident[:], iot_i[:], 0, None, op0=ALU.is_equal)
            nc.vector.tensor_scalar(mask[:], iot_m[:], 0, None, op0=ALU.is_equal)
            nc.vector.tensor_scalar(mask_b[:], iot_m[:], -_E, None, op0=ALU.is_equal)
            nc.vector.tensor_add(mask[:], mask[:], mask_b[:])
            maskh = cpool.tile([_P, _E], bf16, tag="maskh")
            maskq = cpool.tile([_P, _E], bf16, tag="maskq")
            nc.vector.tensor_scalar_mul(maskh[:], mask[:], HALF_SQRT)
            nc.vector.tensor_scalar_mul(maskq[:], mask[:], 0.5)

            # PE HAM warm-up: dummy matmuls on a zeroed tile, result never
            # read. Runs under the first input DMA's shadow and releases
            # the PE clock gate before the real transposes arrive.
            wmov = cpool.tile([_P, _P], bf16, tag="wmov")
            nc.gpsimd.memset(wmov[:], 0.0)
            # full-bank PSUM tile: start=True clears the whole bank, so no
            # other tile may share it
            wps = wpool.tile([_P, 512], f32, tag="wps")
            for _ in range(_WARM_MMS):
                nc.tensor.matmul(wps[:, :_P], wmov[:], ident[:], start=True, stop=True)

            for c in range(_NCHUNK):
                rows = slice(c * _P, (c + 1) * _P)
                slices = _SLICES[c]
                nblk_total = sum(n // 2 for n in slices)
                # full-bank PSUM accumulators (see wps comment)
                s_t = accpool.tile([_P, 512], f32, tag="s")
                q_t = accpool.tile([_P, 512], f32, tag="q")
                s_ps = s_t[:, :_E]
                q_ps = q_t[:, :_E]
                blk0 = 0
                col0 = 0
                for si, n in enumerate(slices):
                    w = n * _E
                    nblk = n // 2
                    xt = xpool.tile([_P, w], f32, tag=f"x{n}")
                    nc.sync.dma_start(out=xt[:], in_=x[rows, col0 : col0 + w])
                    xb = xbpool.tile([_P, w], bf16, tag=f"xb{n}")
                    nc.vector.tensor_copy(xb[:], xt[:])
                    pt = ptpool.tile([_P, 8 * _P], bf16, tag="pt")
                    for j in range(nblk):
                        nc.tensor.transpose(
                            pt[:, j * _P : (j + 1) * _P],
                            xb[:, j * _P : (j + 1) * _P],
                            ident[:],
                        )
                    # copyback split between DVE and ACT for balance
                    xT = xtpool.tile([_P, w], bf16, tag=f"xT{n}")
                    h = (nblk // 2) * _P
                    if h > 0:
                        nc.vector.tensor_copy(xT[:, :h], pt[:, :h])
                    if h < w:
                        nc.scalar.activation(xT[:, h:w], pt[:, h:w], mybir.ActivationFunctionType.Copy)
                    sq = sqpool.tile([_P, w], bf16, tag=f"sq{n}")
                    nc.scalar.activation(sq[:], pt[:, :w], SQ)
                    # grouped matmuls: one stationary load per chain per slice
                    for j in range(nblk):
                        kk = blk0 + j
                        bcols = slice(j * _P, (j + 1) * _P)
                        nc.tensor.matmul(
                            s_ps,
                            xT[:, bcols],
                            maskh[:],
                            start=(kk == 0),
                            stop=(kk == nblk_total - 1),
                        )
                    for j in range(nblk):
                        kk = blk0 + j
                        bcols = slice(j * _P, (j + 1) * _P)
                        nc.tensor.matmul(
                            q_ps,
                            sq[:, bcols],
                            maskq[:],
                            start=(kk == 0),
                            stop=(kk == nblk_total - 1),
                        )
                    blk0 += nblk
                    col0 += w

                # res = 0.5*s^2 - 0.5*q
                m2 = spool.tile([_P, _E], f32, tag=f"m2_{c}")
                res = spool.tile([_P, _E], f32, tag=f"res_{c}")
                nc.scalar.activation(m2[:], s_ps, SQ)
                nc.vector.tensor_sub(res[:], m2[:], q_ps)
                nc.sync.dma_start(out=out[c], in_=res[:])
    nc.compile()
    return nc


_WALRUS_EXTRA = ["--skip-pass=expand_all_engine_final_pre_codegen"]


def _patch_walrus():
    """Hook to append extra walrus_driver args (e.g. --max-sem-num to cap
    the one-event-sem-op-per-semaphore zeroing postamble)."""
    from concourse import bass_utils

    if getattr(bass_utils, "_walrus_patched", False):
        return
    real_run = bass_utils.run_command

    def run2(cmd, **kw):
        if cmd and "walrus_driver" in str(cmd[0]):
            cmd = list(cmd) + _WALRUS_EXTRA
        return real_run(cmd, **kw)

    bass_utils.run_command = run2
    bass_utils._walrus_patched = True


def _run(in_maps, **kwargs):
    from concourse.bass_utils import run_bass_kernel_spmd

    _patch_walrus()
    nc = _build()
    return run_bass_kernel_spmd(nc, in_maps, core_ids=list(range(_NCORES)), **kwargs)


def _shard(inputs: np.ndarray):
    x = np.ascontiguousarray(
        np.asarray(inputs, dtype=np.float32).reshape(_B, _ROW)
    )
    return [
        {"inputs": np.ascontiguousarray(x[i * _BS : (i + 1) * _BS])}
        for i in range(_NCORES)
    ]


def kernel(
    inputs: np.ndarray,
    weight_attention: np.ndarray = None,
    weight_projection: np.ndarray = None,
    weight_bias: np.ndarray = None,
) -> np.ndarray:
    # weights are dead code (softmax over a size-1 axis == 1.0)
    res = _run(_shard(inputs))
    return np.concatenate(
        [r["out"].reshape(_BS, _E) for r in res.results], axis=0
    )
